# revision 2
# baseline (speedup 1.0000x reference)
"""Bass/Trainium2 kernel for nn_KeyValueAttention (B=8, QL=1024, SL=2048).

Strategy: data-parallel over the batch dim — core b computes batch element b.
All matmuls run in bf16 (f32 PSUM accumulation):
  KT = (s@Wk+bk).T, V = s@Wv+bv, QPT = ((q@Wq+bq)/sqrt(QK)).T   (activations
  enter the PE transposed via bf16 DMA-transpose through a DRAM stripe buffer)
  score = QPT.T @ KT + ones x maskneg (mask folded in as a K=1 matmul)
  softmax: exp on ScalarE with fused row-sum accumulation, no max subtraction
  (scores are O(1)), masked lanes get exp(-1e9) == 0
  attn_w = e * recip(sum)  (f32 to DRAM; bf16 copy round-trips through DRAM
  and comes back DMA-transposed for the AV matmul)
  aoT = (attn_w @ V).T ; attn_out = aoT.T @ Wo + bo
"""

import sys

if "/opt/trn_rl_repo" not in sys.path:
    sys.path.insert(0, "/opt/trn_rl_repo")

from contextlib import ExitStack

import numpy as np

import concourse.mybir as mybir
import concourse.tile as tile
from concourse import bacc
from concourse import bass_utils

F32 = mybir.dt.float32
BF16 = mybir.dt.bfloat16
I32 = mybir.dt.int32

B = 8
QL, SL = 1024, 2048
QIN, KVIN = 1024, 1024
QK, VD, OUT = 512, 512, 1024
P = 128
NEGBIG = -1e9

KO_Q = QIN // P   # 8  contraction chunks for q projections
KO_S = KVIN // P  # 8  contraction chunks for s projections
HC = QK // P      # 4  qk_dim chunks
VC = VD // P      # 4  v_dim chunks
JC = SL // P      # 16 key chunks of 128
JN = SL // 512    # 4  key chunks of 512
IC = QL // P      # 8  query chunks of 128
IB = QL // 512    # 2  query blocks of 512
ON = OUT // 512   # 2  output chunks of 512
SCALE = float(1.0 / np.sqrt(QK))

_NC_CACHE = {}


def _build():
    nc = bacc.Bacc("TRN2", target_bir_lowering=False, debug=False,
                   enable_asserts=True, num_devices=B)
    q = nc.dram_tensor("q", [QL, QIN], F32, kind="ExternalInput").ap()
    s = nc.dram_tensor("s", [SL, KVIN], F32, kind="ExternalInput").ap()
    mask = nc.dram_tensor("mask", [1, SL], I32, kind="ExternalInput").ap()
    Wq = nc.dram_tensor("Wq", [QIN, QK], F32, kind="ExternalInput").ap()
    bq = nc.dram_tensor("bq", [QK], F32, kind="ExternalInput").ap()
    Wk = nc.dram_tensor("Wk", [KVIN, QK], F32, kind="ExternalInput").ap()
    bk = nc.dram_tensor("bk", [QK], F32, kind="ExternalInput").ap()
    Wv = nc.dram_tensor("Wv", [KVIN, VD], F32, kind="ExternalInput").ap()
    bv = nc.dram_tensor("bv", [VD], F32, kind="ExternalInput").ap()
    Wo = nc.dram_tensor("Wo", [VD, OUT], F32, kind="ExternalInput").ap()
    bo = nc.dram_tensor("bo", [OUT], F32, kind="ExternalInput").ap()
    attn_w = nc.dram_tensor("attn_w", [QL, SL], F32, kind="ExternalOutput").ap()
    attn_out = nc.dram_tensor("attn_out", [QL, OUT], F32, kind="ExternalOutput").ap()

    # DRAM scratch: column-stripe-major bf16 copies feeding the DMA-transposes
    qbf = nc.dram_tensor("qbf", [KO_Q, QL, P], BF16, kind="Internal").ap()
    sbf = nc.dram_tensor("sbf", [KO_S, SL, P], BF16, kind="Internal").ap()
    awbf = nc.dram_tensor("awbf", [QL, SL], BF16, kind="Internal").ap()

    with tile.TileContext(nc) as tc, ExitStack() as ctx:
        singles = ctx.enter_context(tc.tile_pool(name="singles", bufs=1))
        persist = ctx.enter_context(tc.tile_pool(name="persist", bufs=1))
        psum_small = ctx.enter_context(
            tc.tile_pool(name="psum_small", bufs=4, space="PSUM"))
        psum_score = ctx.enter_context(
            tc.tile_pool(name="psum_score", bufs=2, space="PSUM"))

        # ---------- constants / small inputs ----------
        bq_sb = singles.tile([P, HC], F32)
        nc.sync.dma_start(out=bq_sb, in_=bq.rearrange("(c p) -> p c", p=P))
        bq_s = singles.tile([P, HC], F32)
        nc.vector.tensor_scalar_mul(bq_s, bq_sb, SCALE)
        bk_sb = singles.tile([P, HC], F32)
        nc.sync.dma_start(out=bk_sb, in_=bk.rearrange("(c p) -> p c", p=P))
        bv_sb = singles.tile([P, VD], F32)
        nc.gpsimd.dma_start(out=bv_sb, in_=bv[None, :].to_broadcast((P, VD)))
        bo_sb = singles.tile([P, OUT], F32)
        nc.gpsimd.dma_start(out=bo_sb, in_=bo[None, :].to_broadcast((P, OUT)))

        maskneg = singles.tile([1, SL], BF16)
        ones_row = singles.tile([1, P], BF16)
        nc.vector.memset(ones_row, 1.0)

        # persistent products of the projection stage
        KT = persist.tile([P, HC, SL], BF16)
        V = persist.tile([P, JC, VD], BF16)
        QPT = persist.tile([P, HC, QL], BF16)
        aoT = persist.tile([P, VC, QL], BF16)
        Wo_sb = persist.tile([P, VC, OUT], BF16)
        nc.gpsimd.dma_start(out=Wo_sb, in_=Wo.rearrange("(ko p) n -> p ko n", p=P))

        with tc.tile_pool(name="trans", bufs=1) as trans:
            mask_sb = trans.tile([1, SL], I32)
            nc.sync.dma_start(out=mask_sb, in_=mask[:, :])
            maskf = trans.tile([1, SL], F32)
            nc.vector.tensor_copy(maskf, mask_sb)
            nc.vector.tensor_scalar(maskneg, maskf, -NEGBIG, NEGBIG,
                                    mybir.AluOpType.mult, mybir.AluOpType.add)

            Wq_sb = trans.tile([P, KO_Q, QK], BF16)
            nc.gpsimd.dma_start(out=Wq_sb, in_=Wq.rearrange("(ko p) n -> p ko n", p=P))
            Wk_sb = trans.tile([P, KO_S, QK], BF16)
            nc.gpsimd.dma_start(out=Wk_sb, in_=Wk.rearrange("(ko p) n -> p ko n", p=P))
            Wv_sb = trans.tile([P, KO_S, VD], BF16)
            nc.gpsimd.dma_start(out=Wv_sb, in_=Wv.rearrange("(ko p) n -> p ko n", p=P))

            # ---------- transposed activations ----------
            qT = trans.tile([P, KO_Q, QL], BF16)
            for ko in range(KO_Q):
                nc.gpsimd.dma_start(out=qbf[ko], in_=q[:, ko * P:(ko + 1) * P])
                nc.sync.dma_start_transpose(qT[:, ko, :], qbf[ko])
            sT = trans.tile([P, KO_S, SL], BF16)
            for ko in range(KO_S):
                nc.gpsimd.dma_start(out=sbf[ko], in_=s[:, ko * P:(ko + 1) * P])
                nc.sync.dma_start_transpose(sT[:, ko, :], sbf[ko])

            # ---------- projections ----------
            for hc in range(HC):
                for jn in range(JN):
                    ps = psum_small.tile([P, 512], F32, tag="ps")
                    for kc in range(KO_S):
                        nc.tensor.matmul(
                            ps, Wk_sb[:, kc, hc * P:(hc + 1) * P],
                            sT[:, kc, jn * 512:(jn + 1) * 512],
                            start=(kc == 0), stop=(kc == KO_S - 1))
                    nc.scalar.activation(
                        KT[:, hc, jn * 512:(jn + 1) * 512], ps,
                        mybir.ActivationFunctionType.Identity,
                        bias=bk_sb[:, hc:hc + 1])
            for jc in range(JC):
                ps = psum_small.tile([P, VD], F32, tag="ps")
                for kc in range(KO_S):
                    nc.tensor.matmul(
                        ps, sT[:, kc, jc * P:(jc + 1) * P], Wv_sb[:, kc, :],
                        start=(kc == 0), stop=(kc == KO_S - 1))
                nc.vector.tensor_tensor(V[:, jc, :], ps, bv_sb,
                                        mybir.AluOpType.add)
            for hc in range(HC):
                for ib in range(IB):
                    ps = psum_small.tile([P, 512], F32, tag="ps")
                    for kc in range(KO_Q):
                        nc.tensor.matmul(
                            ps, Wq_sb[:, kc, hc * P:(hc + 1) * P],
                            qT[:, kc, ib * 512:(ib + 1) * 512],
                            start=(kc == 0), stop=(kc == KO_Q - 1))
                    nc.scalar.activation(
                        QPT[:, hc, ib * 512:(ib + 1) * 512], ps,
                        mybir.ActivationFunctionType.Identity,
                        bias=bq_s[:, hc:hc + 1], scale=SCALE)

        # ---------- scores + softmax (per 128-query chunk) ----------
        work = ctx.enter_context(tc.tile_pool(name="work", bufs=2))
        for ic in range(IC):
            isl = slice(ic * P, (ic + 1) * P)
            e = work.tile([P, SL], F32, tag="e")
            ssum = work.tile([P, 2], F32, tag="ssum", bufs=4)
            for half in range(2):
                ps_s = psum_score.tile([P, 1024], F32)
                for jh in range(2):
                    jn = half * 2 + jh
                    psl = slice(jh * 512, (jh + 1) * 512)
                    sl512 = slice(jn * 512, (jn + 1) * 512)
                    nc.tensor.matmul(ps_s[:, psl], ones_row, maskneg[:, sl512],
                                     start=True, stop=False)
                    for hc in range(HC):
                        nc.tensor.matmul(ps_s[:, psl], QPT[:, hc, isl],
                                         KT[:, hc, sl512],
                                         start=False, stop=(hc == HC - 1))
                nc.scalar.activation(
                    e[:, half * 1024:(half + 1) * 1024], ps_s,
                    mybir.ActivationFunctionType.Exp,
                    accum_out=ssum[:, half:half + 1])
            r = work.tile([P, 1], F32, tag="r", bufs=4)
            nc.vector.reduce_sum(r, ssum, axis=mybir.AxisListType.X)
            nc.vector.reciprocal(r, r)
            nc.vector.tensor_scalar_mul(e, e, r)
            nc.sync.dma_start(out=attn_w[isl, :], in_=e)
            aw_bf = work.tile([P, SL], BF16, tag="awb")
            nc.scalar.activation(aw_bf, e, mybir.ActivationFunctionType.Copy)
            nc.sync.dma_start(out=awbf[isl, :], in_=aw_bf)

        # ---------- attention output, per 512-query block ----------
        for ib in range(IB):
            bsl = slice(ib * 512, (ib + 1) * 512)
            awT = work.tile([P, JC, 512], BF16, tag="awT")
            for jo in range(JC):
                nc.sync.dma_start_transpose(awT[:, jo, :],
                                            awbf[bsl, jo * P:(jo + 1) * P])
            for hc in range(VC):
                ps = psum_small.tile([P, 512], F32, tag="ps")
                for jc in range(JC):
                    nc.tensor.matmul(ps, V[:, jc, hc * P:(hc + 1) * P],
                                     awT[:, jc, :],
                                     start=(jc == 0), stop=(jc == JC - 1))
                nc.scalar.activation(aoT[:, hc, bsl], ps,
                                     mybir.ActivationFunctionType.Copy)

        # ---------- output projection ----------
        for ic in range(IC):
            isl = slice(ic * P, (ic + 1) * P)
            out_sb = work.tile([P, OUT], F32, tag="out")
            for on in range(ON):
                osl = slice(on * 512, (on + 1) * 512)
                ps = psum_small.tile([P, 512], F32, tag="ps")
                for hc in range(VC):
                    nc.tensor.matmul(ps, aoT[:, hc, isl], Wo_sb[:, hc, osl],
                                     start=(hc == 0), stop=(hc == VC - 1))
                nc.vector.tensor_tensor(out_sb[:, osl], ps, bo_sb[:, osl],
                                        mybir.AluOpType.add)
            nc.sync.dma_start(out=attn_out[isl, :], in_=out_sb)

    nc.compile()
    return nc


def _get_nc():
    if "nc" not in _NC_CACHE:
        _NC_CACHE["nc"] = _build()
    return _NC_CACHE["nc"]


def kernel(q, s, mask, Wq, bq, Wk, bk, Wv, bv, Wo, bo):
    q = np.asarray(q, np.float32)
    s = np.asarray(s, np.float32)
    mask = np.asarray(mask, np.int32)
    shared = {
        "Wq": np.asarray(Wq, np.float32), "bq": np.asarray(bq, np.float32),
        "Wk": np.asarray(Wk, np.float32), "bk": np.asarray(bk, np.float32),
        "Wv": np.asarray(Wv, np.float32), "bv": np.asarray(bv, np.float32),
        "Wo": np.asarray(Wo, np.float32), "bo": np.asarray(bo, np.float32),
    }
    nc = _get_nc()
    in_maps = [dict(q=q[b], s=s[b], mask=mask[b], **shared) for b in range(B)]
    res = bass_utils.run_bass_kernel_spmd(nc, in_maps, core_ids=list(range(B)))
    attn_w = np.stack([res.results[b]["attn_w"] for b in range(B)])
    attn_out = np.stack([res.results[b]["attn_out"] for b in range(B)])
    return attn_w, attn_out


# revision 3
# speedup vs baseline: 1.1095x; 1.1095x over previous
"""Bass/Trainium2 kernel for nn_KeyValueAttention (B=8, QL=1024, SL=2048).

Data-parallel over batch: core b computes batch element b. bf16 matmuls with
f32 PSUM accumulation. Two decoupled streams share the PE:
  attn_w stream: score = QPT.T@KT (+ ones x maskneg K=1 matmul), exp on ACT
    with fused row-sum, normalize on DVE, f32 out.
  attn_out stream: scoreT = KT.T@QPT computed directly (transposed), exp with
    per-partition mask bias -> unnormalized expT; AV matmul; output projection;
    the softmax normalizer r folds in after the Wo matmul.
q/s reach the PE transposed via: HWDGE f32 load -> DVE bf16 cast -> HWDGE
store -> HWDGE DMA-transpose (xbar) back to SBUF.
"""

import sys

if "/opt/trn_rl_repo" not in sys.path:
    sys.path.insert(0, "/opt/trn_rl_repo")

from contextlib import ExitStack

import numpy as np

import concourse.mybir as mybir
import concourse.tile as tile
from concourse import bacc
from concourse import bass_utils

F32 = mybir.dt.float32
BF16 = mybir.dt.bfloat16
I32 = mybir.dt.int32

B = 8
QL, SL = 1024, 2048
QIN, KVIN = 1024, 1024
QK, VD, OUT = 512, 512, 1024
P = 128
NEGBIG = -1e9

KO_Q = QIN // P   # 8
KO_S = KVIN // P  # 8
HC = QK // P      # 4
VC = VD // P      # 4
JC = SL // P      # 16
JN = SL // 512    # 4
IC = QL // P      # 8
IB = QL // 512    # 2
ON = OUT // 512   # 2
SCALE = float(1.0 / np.sqrt(QK))

_NC_CACHE = {}


def _build():
    nc = bacc.Bacc("TRN2", target_bir_lowering=False, debug=False,
                   enable_asserts=True, num_devices=B)
    q = nc.dram_tensor("q", [QL, QIN], F32, kind="ExternalInput").ap()
    s = nc.dram_tensor("s", [SL, KVIN], F32, kind="ExternalInput").ap()
    mask = nc.dram_tensor("mask", [1, SL], I32, kind="ExternalInput").ap()
    Wq = nc.dram_tensor("Wq", [QIN, QK], F32, kind="ExternalInput").ap()
    bq = nc.dram_tensor("bq", [QK], F32, kind="ExternalInput").ap()
    Wk = nc.dram_tensor("Wk", [KVIN, QK], F32, kind="ExternalInput").ap()
    bk = nc.dram_tensor("bk", [QK], F32, kind="ExternalInput").ap()
    Wv = nc.dram_tensor("Wv", [KVIN, VD], F32, kind="ExternalInput").ap()
    bv = nc.dram_tensor("bv", [VD], F32, kind="ExternalInput").ap()
    Wo = nc.dram_tensor("Wo", [VD, OUT], F32, kind="ExternalInput").ap()
    bo = nc.dram_tensor("bo", [OUT], F32, kind="ExternalInput").ap()
    attn_w = nc.dram_tensor("attn_w", [QL, SL], F32, kind="ExternalOutput").ap()
    attn_out = nc.dram_tensor("attn_out", [QL, OUT], F32, kind="ExternalOutput").ap()

    # DRAM bounce buffers for the bf16 transposes (row-major natural layout)
    qbf = nc.dram_tensor("qbf", [QL, QIN], BF16, kind="Internal").ap()
    sbf = nc.dram_tensor("sbf", [SL, KVIN], BF16, kind="Internal").ap()

    q3 = q.rearrange("(io p) k -> p io k", p=P)
    s3 = s.rearrange("(io p) k -> p io k", p=P)
    qbf3 = qbf.rearrange("(io p) k -> p io k", p=P)
    sbf3 = sbf.rearrange("(io p) k -> p io k", p=P)

    with tile.TileContext(nc) as tc, ExitStack() as ctx:
        singles = ctx.enter_context(tc.tile_pool(name="singles", bufs=1))
        persist = ctx.enter_context(tc.tile_pool(name="persist", bufs=1))
        psum_small = ctx.enter_context(
            tc.tile_pool(name="psum_small", bufs=4, space="PSUM"))
        psum_score = ctx.enter_context(
            tc.tile_pool(name="psum_score", bufs=2, space="PSUM"))

        # ---------- constants ----------
        bq_sb = singles.tile([P, HC], F32)
        nc.sync.dma_start(out=bq_sb, in_=bq.rearrange("(c p) -> p c", p=P))
        bq_s = singles.tile([P, HC], F32)
        nc.vector.tensor_scalar_mul(bq_s, bq_sb, SCALE)
        bk_sb = singles.tile([P, HC], F32)
        nc.sync.dma_start(out=bk_sb, in_=bk.rearrange("(c p) -> p c", p=P))
        bv_sb = singles.tile([P, VD], F32)
        nc.gpsimd.dma_start(out=bv_sb, in_=bv[None, :].to_broadcast((P, VD)))
        bo_sb = singles.tile([P, OUT], F32)
        nc.gpsimd.dma_start(out=bo_sb, in_=bo[None, :].to_broadcast((P, OUT)))

        # mask row form (for the K=1 score matmul) ...
        maskneg = singles.tile([1, SL], BF16)
        mask_sb = singles.tile([1, SL], I32)
        nc.sync.dma_start(out=mask_sb, in_=mask[:, :])
        maskf = singles.tile([1, SL], F32)
        nc.vector.tensor_copy(maskf, mask_sb)
        nc.vector.tensor_scalar(maskneg, maskf, -NEGBIG, NEGBIG,
                                mybir.AluOpType.mult, mybir.AluOpType.add)
        ones_row = singles.tile([1, P], BF16)
        nc.vector.memset(ones_row, 1.0)
        # ... and column form (per-partition exp bias for the scoreT stream)
        maskc_i = singles.tile([P, JC], I32)
        nc.sync.dma_start(out=maskc_i, in_=mask[0].rearrange("(jo p) -> p jo", p=P))
        maskc_f = singles.tile([P, JC], F32)
        nc.vector.tensor_copy(maskc_f, maskc_i)
        maskneg_c = singles.tile([P, JC], F32)
        nc.vector.tensor_scalar(maskneg_c, maskc_f, -NEGBIG, NEGBIG,
                                mybir.AluOpType.mult, mybir.AluOpType.add)

        # persistent projection products
        KT = persist.tile([P, HC, SL], BF16)
        V = persist.tile([P, JC, VD], BF16)
        QPT = persist.tile([P, HC, QL], BF16)
        awT = persist.tile([P, JC, QL], BF16)
        aoT = persist.tile([P, VC, QL], BF16)
        r_all = persist.tile([P, IC], F32)
        Wo_sb = persist.tile([P, VC, OUT], BF16)
        nc.gpsimd.dma_start(out=Wo_sb, in_=Wo.rearrange("(ko p) n -> p ko n", p=P))

        with tc.tile_pool(name="trans", bufs=1) as trans:
            Wq_sb = trans.tile([P, KO_Q, QK], BF16)
            nc.gpsimd.dma_start(out=Wq_sb, in_=Wq.rearrange("(ko p) n -> p ko n", p=P))
            Wk_sb = trans.tile([P, KO_S, QK], BF16)
            nc.gpsimd.dma_start(out=Wk_sb, in_=Wk.rearrange("(ko p) n -> p ko n", p=P))
            Wv_sb = trans.tile([P, KO_S, VD], BF16)
            nc.gpsimd.dma_start(out=Wv_sb, in_=Wv.rearrange("(ko p) n -> p ko n", p=P))

            # f32 load -> DVE bf16 cast -> store -> xbar transpose back
            with tc.tile_pool(name="castp", bufs=3) as castp:
                for io in range(QL // P):
                    cf = castp.tile([P, QIN], F32, tag="cf")
                    nc.sync.dma_start(out=cf, in_=q3[:, io, :])
                    cb = castp.tile([P, QIN], BF16, tag="cb")
                    nc.vector.tensor_copy(cb, cf)
                    nc.sync.dma_start(out=qbf3[:, io, :], in_=cb)
                for io in range(SL // P):
                    cf = castp.tile([P, KVIN], F32, tag="cf")
                    nc.sync.dma_start(out=cf, in_=s3[:, io, :])
                    cb = castp.tile([P, KVIN], BF16, tag="cb")
                    nc.vector.tensor_copy(cb, cf)
                    nc.sync.dma_start(out=sbf3[:, io, :], in_=cb)

            qT = trans.tile([P, KO_Q, QL], BF16)
            for ko in range(KO_Q):
                nc.sync.dma_start_transpose(qT[:, ko, :], qbf[:, ko * P:(ko + 1) * P])
            sT = trans.tile([P, KO_S, SL], BF16)
            for ko in range(KO_S):
                nc.sync.dma_start_transpose(sT[:, ko, :], sbf[:, ko * P:(ko + 1) * P])

            # ---------- projections ----------
            for hc in range(HC):
                for jn in range(JN):
                    ps = psum_small.tile([P, 512], F32, tag="ps")
                    for kc in range(KO_S):
                        nc.tensor.matmul(
                            ps, Wk_sb[:, kc, hc * P:(hc + 1) * P],
                            sT[:, kc, jn * 512:(jn + 1) * 512],
                            start=(kc == 0), stop=(kc == KO_S - 1))
                    nc.vector.tensor_scalar(
                        KT[:, hc, jn * 512:(jn + 1) * 512], ps,
                        bk_sb[:, hc:hc + 1], None, mybir.AluOpType.add)
                for ib in range(IB):
                    ps = psum_small.tile([P, 512], F32, tag="ps")
                    for kc in range(KO_Q):
                        nc.tensor.matmul(
                            ps, Wq_sb[:, kc, hc * P:(hc + 1) * P],
                            qT[:, kc, ib * 512:(ib + 1) * 512],
                            start=(kc == 0), stop=(kc == KO_Q - 1))
                    nc.vector.tensor_scalar(
                        QPT[:, hc, ib * 512:(ib + 1) * 512], ps,
                        SCALE, bq_s[:, hc:hc + 1],
                        mybir.AluOpType.mult, mybir.AluOpType.add)
            for jc in range(JC):
                ps = psum_small.tile([P, VD], F32, tag="ps")
                for kc in range(KO_S):
                    nc.tensor.matmul(
                        ps, sT[:, kc, jc * P:(jc + 1) * P], Wv_sb[:, kc, :],
                        start=(kc == 0), stop=(kc == KO_S - 1))
                nc.vector.tensor_tensor(V[:, jc, :], ps, bv_sb,
                                        mybir.AluOpType.add)

        # ---------- attn_out stream: transposed scores -> expT ----------
        for jc in range(JC):
            for ib in range(IB):
                ps = psum_small.tile([P, 512], F32, tag="ps")
                for hc in range(HC):
                    nc.tensor.matmul(ps, KT[:, hc, jc * P:(jc + 1) * P],
                                     QPT[:, hc, ib * 512:(ib + 1) * 512],
                                     start=(hc == 0), stop=(hc == HC - 1))
                nc.scalar.activation(awT[:, jc, ib * 512:(ib + 1) * 512], ps,
                                     mybir.ActivationFunctionType.Exp,
                                     bias=maskneg_c[:, jc:jc + 1])

        # ---------- attn_w stream: scores + softmax per 128-query chunk ----
        work = ctx.enter_context(tc.tile_pool(name="work", bufs=2))
        for ic in range(IC):
            isl = slice(ic * P, (ic + 1) * P)
            e = work.tile([P, SL], F32, tag="e")
            ssum = work.tile([P, 2], F32, tag="ssum", bufs=4)
            for half in range(2):
                ps_s = psum_score.tile([P, 1024], F32)
                for jh in range(2):
                    jn = half * 2 + jh
                    psl = slice(jh * 512, (jh + 1) * 512)
                    sl512 = slice(jn * 512, (jn + 1) * 512)
                    nc.tensor.matmul(ps_s[:, psl], ones_row, maskneg[:, sl512],
                                     start=True, stop=False)
                    for hc in range(HC):
                        nc.tensor.matmul(ps_s[:, psl], QPT[:, hc, isl],
                                         KT[:, hc, sl512],
                                         start=False, stop=(hc == HC - 1))
                nc.scalar.activation(
                    e[:, half * 1024:(half + 1) * 1024], ps_s,
                    mybir.ActivationFunctionType.Exp,
                    accum_out=ssum[:, half:half + 1])
            nc.vector.reduce_sum(r_all[:, ic:ic + 1], ssum,
                                 axis=mybir.AxisListType.X)
            nc.vector.reciprocal(r_all[:, ic:ic + 1], r_all[:, ic:ic + 1])
            nc.vector.tensor_scalar_mul(e, e, r_all[:, ic:ic + 1])
            nc.sync.dma_start(out=attn_w[isl, :], in_=e)

        # ---------- AV matmul (uses unnormalized expT) ----------
        for ib in range(IB):
            bsl = slice(ib * 512, (ib + 1) * 512)
            for hc in range(VC):
                ps = psum_small.tile([P, 512], F32, tag="ps")
                for jc in range(JC):
                    nc.tensor.matmul(ps, V[:, jc, hc * P:(hc + 1) * P],
                                     awT[:, jc, bsl],
                                     start=(jc == 0), stop=(jc == JC - 1))
                nc.vector.tensor_copy(aoT[:, hc, bsl], ps)

        # ---------- output projection (normalizer folds in here) ----------
        for ic in range(IC):
            isl = slice(ic * P, (ic + 1) * P)
            out_sb = work.tile([P, OUT], F32, tag="out")
            for on in range(ON):
                osl = slice(on * 512, (on + 1) * 512)
                ps = psum_small.tile([P, 512], F32, tag="ps")
                for hc in range(VC):
                    nc.tensor.matmul(ps, aoT[:, hc, isl], Wo_sb[:, hc, osl],
                                     start=(hc == 0), stop=(hc == VC - 1))
                nc.vector.tensor_scalar_mul(out_sb[:, osl], ps,
                                            r_all[:, ic:ic + 1])
                nc.vector.tensor_tensor(out_sb[:, osl], out_sb[:, osl],
                                        bo_sb[:, osl], mybir.AluOpType.add)
            nc.sync.dma_start(out=attn_out[isl, :], in_=out_sb)

    nc.compile()
    return nc


def _get_nc():
    if "nc" not in _NC_CACHE:
        _NC_CACHE["nc"] = _build()
    return _NC_CACHE["nc"]


def kernel(q, s, mask, Wq, bq, Wk, bk, Wv, bv, Wo, bo):
    q = np.asarray(q, np.float32)
    s = np.asarray(s, np.float32)
    mask = np.asarray(mask, np.int32)
    shared = {
        "Wq": np.asarray(Wq, np.float32), "bq": np.asarray(bq, np.float32),
        "Wk": np.asarray(Wk, np.float32), "bk": np.asarray(bk, np.float32),
        "Wv": np.asarray(Wv, np.float32), "bv": np.asarray(bv, np.float32),
        "Wo": np.asarray(Wo, np.float32), "bo": np.asarray(bo, np.float32),
    }
    nc = _get_nc()
    in_maps = [dict(q=q[b], s=s[b], mask=mask[b], **shared) for b in range(B)]
    res = bass_utils.run_bass_kernel_spmd(nc, in_maps, core_ids=list(range(B)))
    attn_w = np.stack([res.results[b]["attn_w"] for b in range(B)])
    attn_out = np.stack([res.results[b]["attn_out"] for b in range(B)])
    return attn_w, attn_out


# revision 7
# speedup vs baseline: 1.1468x; 1.0337x over previous
"""Bass/Trainium2 kernel for nn_KeyValueAttention (B=8, QL=1024, SL=2048).

Data-parallel over batch: core b computes batch element b. bf16 matmuls with
f32 PSUM accumulation. Two decoupled streams share the PE:
  attn_w stream: score = QPT.T@KT (+ ones x maskneg K=1 matmul), exp on ACT
    with fused row-sum, normalize on DVE, f32 out.
  attn_out stream: scoreT = KT.T@QPT computed directly (transposed), exp with
    per-partition mask bias -> unnormalized expT; AV matmul; output projection;
    the softmax normalizer r folds in after the Wo matmul.
q/s reach the PE transposed via 512-row blocks: HWDGE f32 load -> DVE bf16
cast -> HWDGE store -> per-block HWDGE DMA-transposes (xbar), so the first
projection matmuls start as soon as the first block lands. DMA issue is split
across both HWDGE rings (s-chain+weights on sync, q-chain+outputs on scalar).
"""

import sys

if "/opt/trn_rl_repo" not in sys.path:
    sys.path.insert(0, "/opt/trn_rl_repo")

from contextlib import ExitStack

import numpy as np

import concourse.mybir as mybir
import concourse.tile as tile
from concourse import bacc
from concourse import bass_utils

F32 = mybir.dt.float32
BF16 = mybir.dt.bfloat16
I32 = mybir.dt.int32

B = 8
QL, SL = 1024, 2048
QIN, KVIN = 1024, 1024
QK, VD, OUT = 512, 512, 1024
P = 128
NEGBIG = -1e9

KO_Q = QIN // P   # 8
KO_S = KVIN // P  # 8
HC = QK // P      # 4
VC = VD // P      # 4
JC = SL // P      # 16
JN = SL // 512    # 4
IC = QL // P      # 8
IB = QL // 512    # 2
ON = OUT // 512   # 2
SCALE = float(1.0 / np.sqrt(QK))

_NC_CACHE = {}


def _build():
    nc = bacc.Bacc("TRN2", target_bir_lowering=False, debug=False,
                   enable_asserts=True, num_devices=B)
    q = nc.dram_tensor("q", [QL, QIN], F32, kind="ExternalInput").ap()
    s = nc.dram_tensor("s", [SL, KVIN], F32, kind="ExternalInput").ap()
    mask = nc.dram_tensor("mask", [1, SL], I32, kind="ExternalInput").ap()
    Wq = nc.dram_tensor("Wq", [QIN, QK], F32, kind="ExternalInput").ap()
    bq = nc.dram_tensor("bq", [QK], F32, kind="ExternalInput").ap()
    Wk = nc.dram_tensor("Wk", [KVIN, QK], F32, kind="ExternalInput").ap()
    bk = nc.dram_tensor("bk", [QK], F32, kind="ExternalInput").ap()
    Wv = nc.dram_tensor("Wv", [KVIN, VD], F32, kind="ExternalInput").ap()
    bv = nc.dram_tensor("bv", [VD], F32, kind="ExternalInput").ap()
    Wo = nc.dram_tensor("Wo", [VD, OUT], F32, kind="ExternalInput").ap()
    bo = nc.dram_tensor("bo", [OUT], F32, kind="ExternalInput").ap()
    attn_w = nc.dram_tensor("attn_w", [QL, SL], F32, kind="ExternalOutput").ap()
    attn_out = nc.dram_tensor("attn_out", [QL, OUT], F32, kind="ExternalOutput").ap()

    # DRAM bounce buffers for the bf16 transposes (row-major natural layout)
    qbf = nc.dram_tensor("qbf", [QL, QIN], BF16, kind="Internal").ap()
    sbf = nc.dram_tensor("sbf", [SL, KVIN], BF16, kind="Internal").ap()

    RB = 512                      # row-block for the cast/transpose pipeline
    q4 = q.rearrange("(b p rr) k -> b p rr k", p=P, rr=RB // P)
    s4 = s.rearrange("(b p rr) k -> b p rr k", p=P, rr=RB // P)
    qbf4 = qbf.rearrange("(b p rr) k -> b p rr k", p=P, rr=RB // P)
    sbf4 = sbf.rearrange("(b p rr) k -> b p rr k", p=P, rr=RB // P)

    with tile.TileContext(nc) as tc, ExitStack() as ctx:
        singles = ctx.enter_context(tc.tile_pool(name="singles", bufs=1))
        persist = ctx.enter_context(tc.tile_pool(name="persist", bufs=1))
        psum_small = ctx.enter_context(
            tc.tile_pool(name="psum_small", bufs=4, space="PSUM"))
        psum_score = ctx.enter_context(
            tc.tile_pool(name="psum_score", bufs=2, space="PSUM"))

        # ---------- constants ----------
        bq_sb = singles.tile([P, HC], F32)
        nc.scalar.dma_start(out=bq_sb, in_=bq.rearrange("(c p) -> p c", p=P))
        bq_s = singles.tile([P, HC], F32)
        nc.vector.tensor_scalar_mul(bq_s, bq_sb, SCALE)
        bk_sb = singles.tile([P, HC], F32)
        nc.scalar.dma_start(out=bk_sb, in_=bk.rearrange("(c p) -> p c", p=P))
        bv_sb = singles.tile([P, VD], F32)
        nc.gpsimd.dma_start(out=bv_sb, in_=bv[None, :].to_broadcast((P, VD)))
        bo_sb = singles.tile([P, OUT], F32)
        nc.gpsimd.dma_start(out=bo_sb, in_=bo[None, :].to_broadcast((P, OUT)))

        maskneg = singles.tile([1, SL], BF16)
        ones_row = singles.tile([1, P], BF16)
        nc.vector.memset(ones_row, 1.0)
        maskneg_c = singles.tile([P, JC], F32)

        # persistent projection products (attn_out-stream tiles come later)
        KT = persist.tile([P, HC, SL], BF16)
        V = persist.tile([P, JC, VD], BF16)
        QPT = persist.tile([P, HC, QL], BF16)
        r_all = persist.tile([P, IC], F32)
        Wo_sb = persist.tile([P, VC, OUT], BF16)

        with tc.tile_pool(name="trans", bufs=1) as trans:
            qT = trans.tile([P, KO_Q, QL], BF16)
            sT = trans.tile([P, KO_S, SL], BF16)
            Wq_sb = trans.tile([P, KO_Q, QK], BF16)
            Wk_sb = trans.tile([P, KO_S, QK], BF16)
            Wv_sb = trans.tile([P, KO_S, VD], BF16)

            with tc.tile_pool(name="castp", bufs=1) as castp:
                # weights: HWDGE f32 load + DVE cast (SWDGE is too slow)
                for name, dst, src, ko in (
                        ("wk", Wk_sb, Wk, KO_S), ("wv", Wv_sb, Wv, KO_S),
                        ("wq", Wq_sb, Wq, KO_Q), ("wo", Wo_sb, Wo, VC)):
                    wf = castp.tile([P, ko, dst.shape[2]], F32, tag="wf", bufs=1)
                    nc.sync.dma_start(
                        out=wf, in_=src.rearrange("(c p) n -> p c n", p=P))
                    nc.vector.tensor_copy(dst, wf)

                # s then q: load row-block, cast, store, transpose that block
                for (t4, b4, bounce, tT, nko, nb, eng) in (
                        (s4, sbf4, sbf, sT, KO_S, SL // RB, nc.sync),
                        (q4, qbf4, qbf, qT, KO_Q, QL // RB, nc.scalar)):
                    for blk in range(nb):
                        cf = castp.tile([P, RB // P, t4.shape[3]], F32,
                                        tag="cf", bufs=2)
                        eng.dma_start(out=cf, in_=t4[blk])
                        cb = castp.tile([P, RB // P, t4.shape[3]], BF16,
                                        tag="cb", bufs=2)
                        nc.vector.tensor_copy(cb, cf)
                        eng.dma_start(out=b4[blk], in_=cb)
                        for ko in range(nko):
                            eng.dma_start_transpose(
                                tT[:, ko, blk * RB:(blk + 1) * RB],
                                bounce[blk * RB:(blk + 1) * RB,
                                       ko * P:(ko + 1) * P])

                # ---------- projections ----------
                for jn in range(JN):
                    for hc in range(HC):
                        ps = psum_small.tile([P, 512], F32, tag="ps")
                        for kc in range(KO_S):
                            nc.tensor.matmul(
                                ps, Wk_sb[:, kc, hc * P:(hc + 1) * P],
                                sT[:, kc, jn * 512:(jn + 1) * 512],
                                start=(kc == 0), stop=(kc == KO_S - 1))
                        nc.vector.tensor_scalar(
                            KT[:, hc, jn * 512:(jn + 1) * 512], ps,
                            bk_sb[:, hc:hc + 1], None, mybir.AluOpType.add)
                for jc in range(JC):
                    ps = psum_small.tile([P, VD], F32, tag="ps")
                    for kc in range(KO_S):
                        nc.tensor.matmul(
                            ps, sT[:, kc, jc * P:(jc + 1) * P], Wv_sb[:, kc, :],
                            start=(kc == 0), stop=(kc == KO_S - 1))
                    nc.vector.tensor_tensor(V[:, jc, :], ps, bv_sb,
                                            mybir.AluOpType.add)
                for hc in range(HC):
                    for ib in range(IB):
                        ps = psum_small.tile([P, 512], F32, tag="ps")
                        for kc in range(KO_Q):
                            nc.tensor.matmul(
                                ps, Wq_sb[:, kc, hc * P:(hc + 1) * P],
                                qT[:, kc, ib * 512:(ib + 1) * 512],
                                start=(kc == 0), stop=(kc == KO_Q - 1))
                        nc.vector.tensor_scalar(
                            QPT[:, hc, ib * 512:(ib + 1) * 512], ps,
                            SCALE, bq_s[:, hc:hc + 1],
                            mybir.AluOpType.mult, mybir.AluOpType.add)

        # ---------- attn_out stream: transposed scores -> expT ----------
        late = ctx.enter_context(tc.tile_pool(name="late", bufs=1))
        awT = late.tile([P, JC, QL], BF16)
        aoT = late.tile([P, VC, QL], BF16)
        work = ctx.enter_context(tc.tile_pool(name="work", bufs=2))

        # mask, row form (K=1 matmul rhs) and per-partition column form
        mask_sb = work.tile([1, SL], I32, tag="mask_sb", bufs=1)
        nc.scalar.dma_start(out=mask_sb, in_=mask[:, :])
        maskf = work.tile([1, SL], F32, tag="maskf", bufs=1)
        nc.vector.tensor_copy(maskf, mask_sb)
        nc.vector.tensor_scalar(maskneg, maskf, -NEGBIG, NEGBIG,
                                mybir.AluOpType.mult, mybir.AluOpType.add)
        maskc_i = work.tile([P, JC], I32, tag="maskc_i", bufs=1)
        nc.scalar.dma_start(out=maskc_i,
                            in_=mask[0].rearrange("(jo p) -> p jo", p=P))
        maskc_f = work.tile([P, JC], F32, tag="maskc_f", bufs=1)
        nc.vector.tensor_copy(maskc_f, maskc_i)
        nc.vector.tensor_scalar(maskneg_c, maskc_f, -NEGBIG, NEGBIG,
                                mybir.AluOpType.mult, mybir.AluOpType.add)

        for ib in range(IB):
            bsl = slice(ib * 512, (ib + 1) * 512)
            for jc in range(JC):
                ps = psum_small.tile([P, 512], F32, tag="ps")
                for hc in range(HC):
                    nc.tensor.matmul(ps, KT[:, hc, jc * P:(jc + 1) * P],
                                     QPT[:, hc, bsl],
                                     start=(hc == 0), stop=(hc == HC - 1))
                nc.scalar.activation(awT[:, jc, bsl], ps,
                                     mybir.ActivationFunctionType.Exp,
                                     bias=maskneg_c[:, jc:jc + 1])
            # AV matmul for this query block (uses unnormalized expT)
            for hc in range(VC):
                ps = psum_small.tile([P, 512], F32, tag="ps")
                for jc in range(JC):
                    nc.tensor.matmul(ps, V[:, jc, hc * P:(hc + 1) * P],
                                     awT[:, jc, bsl],
                                     start=(jc == 0), stop=(jc == JC - 1))
                nc.vector.tensor_copy(aoT[:, hc, bsl], ps)

        # ---------- attn_w stream: scores + softmax per 128-query chunk ----
        for ic in range(IC):
            isl = slice(ic * P, (ic + 1) * P)
            e = work.tile([P, SL], F32, tag="e")
            ssum = work.tile([P, 2], F32, tag="ssum", bufs=4)
            for half in range(2):
                ps_s = psum_score.tile([P, 1024], F32)
                for jh in range(2):
                    jn = half * 2 + jh
                    psl = slice(jh * 512, (jh + 1) * 512)
                    sl512 = slice(jn * 512, (jn + 1) * 512)
                    nc.tensor.matmul(ps_s[:, psl], ones_row, maskneg[:, sl512],
                                     start=True, stop=False)
                    for hc in range(HC):
                        nc.tensor.matmul(ps_s[:, psl], QPT[:, hc, isl],
                                         KT[:, hc, sl512],
                                         start=False, stop=(hc == HC - 1))
                nc.scalar.activation(
                    e[:, half * 1024:(half + 1) * 1024], ps_s,
                    mybir.ActivationFunctionType.Exp,
                    accum_out=ssum[:, half:half + 1])
            nc.vector.reduce_sum(r_all[:, ic:ic + 1], ssum,
                                 axis=mybir.AxisListType.X)
            nc.vector.reciprocal(r_all[:, ic:ic + 1], r_all[:, ic:ic + 1])
            nc.vector.tensor_scalar_mul(e, e, r_all[:, ic:ic + 1])
            nc.scalar.dma_start(out=attn_w[isl, :], in_=e)

        # ---------- output projection (normalizer folds in here) ----------
        for ic in range(IC):
            isl = slice(ic * P, (ic + 1) * P)
            out_sb = work.tile([P, OUT], F32, tag="out")
            for on in range(ON):
                osl = slice(on * 512, (on + 1) * 512)
                ps = psum_small.tile([P, 512], F32, tag="ps")
                for hc in range(VC):
                    nc.tensor.matmul(ps, aoT[:, hc, isl], Wo_sb[:, hc, osl],
                                     start=(hc == 0), stop=(hc == VC - 1))
                nc.vector.tensor_scalar_mul(out_sb[:, osl], ps,
                                            r_all[:, ic:ic + 1])
                nc.vector.tensor_tensor(out_sb[:, osl], out_sb[:, osl],
                                        bo_sb[:, osl], mybir.AluOpType.add)
            nc.scalar.dma_start(out=attn_out[isl, :], in_=out_sb)

    nc.compile()
    return nc


def _get_nc():
    if "nc" not in _NC_CACHE:
        _NC_CACHE["nc"] = _build()
    return _NC_CACHE["nc"]


def kernel(q, s, mask, Wq, bq, Wk, bk, Wv, bv, Wo, bo):
    q = np.asarray(q, np.float32)
    s = np.asarray(s, np.float32)
    mask = np.asarray(mask, np.int32)
    shared = {
        "Wq": np.asarray(Wq, np.float32), "bq": np.asarray(bq, np.float32),
        "Wk": np.asarray(Wk, np.float32), "bk": np.asarray(bk, np.float32),
        "Wv": np.asarray(Wv, np.float32), "bv": np.asarray(bv, np.float32),
        "Wo": np.asarray(Wo, np.float32), "bo": np.asarray(bo, np.float32),
    }
    nc = _get_nc()
    in_maps = [dict(q=q[b], s=s[b], mask=mask[b], **shared) for b in range(B)]
    res = bass_utils.run_bass_kernel_spmd(nc, in_maps, core_ids=list(range(B)))
    attn_w = np.stack([res.results[b]["attn_w"] for b in range(B)])
    attn_out = np.stack([res.results[b]["attn_out"] for b in range(B)])
    return attn_w, attn_out
